# revision 16
# baseline (speedup 1.0000x reference)
"""Trainium2 Bass kernel for nn_EnhancedCausalModel.

Computes influence = mean_A[ softmax(p_wo) * (log_softmax(p_wo) - log_softmax(p_with)) ]
where p_with = MLP([obs, actions]) and p_wo is the average of 11 MLP passes
([obs, 0] plus 10 counterfactual uniform action draws, jax threefry key 42).

Strategy (data-parallel over 8 NeuronCores, batch-sharded):
- Activations are feature-major ("transposed"): SBUF tiles are
  [features(partitions), samples(free)]. Host pre-transposes per-core inputs.
- obs @ W1[:256] is computed ONCE per sample tile. Each of the 11 action
  variants adds its small action matmul; the shared (o1+b1) is added either
  by a PE identity-matmul (then a 1-input relu extraction) or by a DVE
  scalar_tensor_tensor (then a cheap SBUF relu) to balance engine load.
- Layer 3 is linear: the 11 "without" variants accumulate in PSUM. Pairs of
  h2 tensors are stacked into [128, F] via SBUF->SBUF DMA so one K=128
  matmul with [W3; W3] handles two variants.
- Softmax/KL stats (Zq, Zw, U) are per-sample partition sums via M=1
  ones-matmuls; U uses the difference-first formulation
  u = e_wo * (p_wo/11 - p_with) to preserve the tiny output's precision.
- All fp32 matmuls use float32r (full rate on TRN2, ~1.5e-4 per-matmul).
"""

import os
import sys
import types
import numpy as np
import ml_dtypes
from contextlib import ExitStack

import concourse.bass as bass
import concourse.tile as tile
from concourse import bacc, mybir
from concourse.bass_utils import run_bass_kernel_spmd

f32 = mybir.dt.float32
f32r = mybir.dt.float32r
bf16 = mybir.dt.bfloat16

# Problem shape (hardcoded per spec)
B, T, N, D, A = 32, 512, 8, 256, 64
NCORES = 8
S = B * T * N // NCORES  # samples per core = 16384
F = 512                  # samples per tile
NT = S // F              # 32 tiles
H1, H2 = 128, 64
NCF = 10                 # counterfactual draws
NAV = NCF + 1            # action variants (real + cf)
NWO = NCF + 1            # "without" variants averaged (zeros + cf)

AV_BF16 = bool(int(os.environ.get("AV_BF16", "0")))
AV_DT = bf16 if AV_BF16 else f32r
AV_NP = ml_dtypes.bfloat16 if AV_BF16 else np.float32

# Variants 0..10 (0 = real actions). First NIADD go through the PE
# identity-add path; the rest through DVE scalar_tensor_tensor.
NIADD = int(os.environ.get("NIADD", "9"))

_CACHE = {}


def _shim_axon_hooks():
    """The image's antenv lacks axon_hooks; provide it so trace=True works."""
    if "antenv.axon_hooks" in sys.modules:
        return
    try:
        import antenv
        import trn_agent_boot.trn_boot as tb
        import concourse.bass_utils as bu

        mod = types.ModuleType("antenv.axon_hooks")
        _hook = [None]
        mod.set_axon_ntff_profile_hook = lambda h: _hook.__setitem__(0, h)
        mod.get_axon_ntff_profile_hook = lambda: _hook[0]
        sys.modules["antenv.axon_hooks"] = mod
        antenv.axon_hooks = mod
        mod.set_axon_ntff_profile_hook(
            tb._ntff_profile_via_ctypes("/opt/axon/libaxon_pjrt.so")
        )
        bu.upload_artifacts = lambda d: d
    except Exception:
        pass


def _build():
    if "nc" in _CACHE:
        return _CACHE["nc"]

    nc = bacc.Bacc("TRN2", target_bir_lowering=False, debug=False,
                   num_devices=NCORES)

    obsT_ap = nc.dram_tensor("obsT", [D, S], f32r, kind="ExternalInput").ap()
    avT_ap = nc.dram_tensor("avT", [NAV, A, S], AV_DT, kind="ExternalInput").ap()
    wobs_ap = nc.dram_tensor("wobs", [128, D], f32r, kind="ExternalInput").ap()
    wact_ap = nc.dram_tensor("wact", [128, H1], AV_DT, kind="ExternalInput").ap()
    w2_ap = nc.dram_tensor("w2", [H1, H2], f32r, kind="ExternalInput").ap()
    w3d_ap = nc.dram_tensor("w3d", [128, A], f32r, kind="ExternalInput").ap()
    ident_ap = nc.dram_tensor("ident", [128, 128], f32r, kind="ExternalInput").ap()
    redv_ap = nc.dram_tensor("redv", [64, 2], f32r, kind="ExternalInput").ap()
    bias_ap = nc.dram_tensor("bias", [128, 3], f32, kind="ExternalInput").ap()
    out_ap = nc.dram_tensor("out", [S], f32, kind="ExternalOutput").ap()

    with tile.TileContext(nc) as tc:
        with ExitStack() as ctx:
            singles = ctx.enter_context(tc.tile_pool(name="singles", bufs=1))
            obs_pool = ctx.enter_context(tc.tile_pool(name="obs", bufs=3))
            av_pool = ctx.enter_context(tc.tile_pool(name="av", bufs=8))
            sb1 = ctx.enter_context(tc.tile_pool(name="sb1", bufs=3))
            h1_pool = ctx.enter_context(tc.tile_pool(name="h1", bufs=6))
            h2_pool = ctx.enter_context(tc.tile_pool(name="h2", bufs=4))
            h2st_pool = ctx.enter_context(tc.tile_pool(name="h2st", bufs=4))
            eu_pool = ctx.enter_context(tc.tile_pool(name="eu", bufs=2))
            ssb_pool = ctx.enter_context(tc.tile_pool(name="ssb", bufs=4))
            tail_pool = ctx.enter_context(tc.tile_pool(name="tail", bufs=1))

            pa1 = ctx.enter_context(tc.tile_pool(name="pa1", bufs=2, space="PSUM"))
            ph2 = ctx.enter_context(tc.tile_pool(name="ph2", bufs=1, space="PSUM"))
            pmisc = ctx.enter_context(tc.tile_pool(name="pmisc", bufs=2, space="PSUM"))

            # --- constants ---
            wobs_s = singles.tile([128, D], f32r)
            nc.sync.dma_start(out=wobs_s[:], in_=wobs_ap[:])
            wact_s = singles.tile([128, H1], AV_DT)
            nc.sync.dma_start(out=wact_s[:], in_=wact_ap[:])
            w2_s = singles.tile([H1, H2], f32r)
            nc.sync.dma_start(out=w2_s[:], in_=w2_ap[:])
            w3d_s = singles.tile([128, A], f32r)  # [W3; W3]
            nc.sync.dma_start(out=w3d_s[:], in_=w3d_ap[:])
            ident_s = singles.tile([128, 128], f32r)
            nc.sync.dma_start(out=ident_s[:], in_=ident_ap[:])
            redv_s = singles.tile([64, 2], f32r)  # c0=+1, c1=+1/64
            nc.sync.dma_start(out=redv_s[:], in_=redv_ap[:])
            bias_s = singles.tile([128, 3], f32)
            nc.sync.dma_start(out=bias_s[:], in_=bias_ap[:])

            # staging for the batched tail
            zq_st = singles.tile([128, 128], f32)
            zw_st = singles.tile([128, 128], f32)
            u_st = singles.tile([128, 128], f32)

            b1_bias = bias_s[:, 0:1]
            b2_bias = bias_s[0:64, 1:2]
            b3_bias = bias_s[0:64, 2:3]

            obsT_r = obsT_ap.rearrange("(c p) s -> p c s", p=128)

            for t in range(NT):
                ts = bass.ts(t, F)

                # --- loads ---
                obs_t = obs_pool.tile([128, 2, F], f32r)
                nc.sync.dma_start(out=obs_t[:], in_=obsT_r[:, :, ts])
                av_tiles = []
                for p in range((NAV + 1) // 2):
                    avp = av_pool.tile([128, F], AV_DT)
                    nc.sync.dma_start(out=avp[0:64, :], in_=avT_ap[2 * p, :, ts])
                    if 2 * p + 1 < NAV:
                        nc.sync.dma_start(
                            out=avp[64:128, :], in_=avT_ap[2 * p + 1, :, ts]
                        )
                    av_tiles.append(avp)

                # --- o1 = W1o.T @ obsT  (shared) ---
                o1t = pa1.tile([128, 2, F], f32, tag="a1")
                o1ps = o1t[:, 0, :]
                nc.tensor.matmul(o1ps, wobs_s[:, 0:128], obs_t[:, 0, :],
                                 start=True, stop=False)
                nc.tensor.matmul(o1ps, wobs_s[:, 128:256], obs_t[:, 1, :],
                                 start=False, stop=True)

                # o1b = o1 + b1 -> SBUF (f32r)
                o1b = sb1.tile([128, F], f32r)
                nc.scalar.activation(o1b[:], o1ps,
                                     mybir.ActivationFunctionType.Identity,
                                     bias=b1_bias)

                # --- h1 for 12 variants ---
                h1_views = [None] * (NAV + 1)  # 0..10 variants, 11 = zeros

                h1z = h1_pool.tile([128, F], f32r, tag="h1s")
                nc.vector.tensor_scalar_max(h1z[:], o1b[:], 0.0)
                h1_views[NAV] = h1z[:]

                ndve = 0
                for p in range((NAV + 1) // 2):
                    vs = [v for v in (2 * p, 2 * p + 1) if v < NAV]
                    paps = pa1.tile([128, 2, F], f32, tag="a1")
                    # a1 matmuls first (row-strip pair overlaps), then the
                    # identity-adds back to back (same weights adjacent).
                    for j, v in enumerate(vs):
                        nc.tensor.matmul(
                            paps[:, j, :],
                            wact_s[64 * j: 64 * (j + 1), :],
                            av_tiles[p][64 * j: 64 * (j + 1), :],
                            start=True,
                            stop=not (v < NIADD),
                        )
                    for j, v in enumerate(vs):
                        if v < NIADD:
                            nc.tensor.matmul(paps[:, j, :], ident_s[:], o1b[:],
                                             start=False, stop=True)
                    if all(v < NIADD for v in vs) and len(vs) == 2:
                        # paired 1-input relu extraction
                        h1p = h1_pool.tile([128, 2, F], f32r, tag="h1p")
                        if p % 2 == 0:
                            nc.scalar.activation(
                                h1p[:], paps[:],
                                mybir.ActivationFunctionType.Relu)
                        else:
                            nc.vector.tensor_scalar(
                                out=h1p[:], in0=paps[:],
                                scalar1=0.0, scalar2=None,
                                op0=mybir.AluOpType.max,
                            )
                        for j, v in enumerate(vs):
                            h1_views[v] = h1p[:, j, :]
                    else:
                        for j, v in enumerate(vs):
                            h1v = h1_pool.tile([128, F], f32r, tag="h1s")
                            if v < NIADD:
                                nc.scalar.activation(
                                    h1v[:], paps[:, j, :],
                                    mybir.ActivationFunctionType.Relu)
                            else:
                                # DVE add of o1b, then cheap SBUF relu
                                pre = h1_pool.tile([128, F], f32r, tag="h1pre")
                                nc.vector.scalar_tensor_tensor(
                                    out=pre[:], in0=paps[:, j, :],
                                    scalar=1.0, in1=o1b[:],
                                    op0=mybir.AluOpType.mult,
                                    op1=mybir.AluOpType.add,
                                )
                                if ndve % 2 == 0:
                                    nc.vector.tensor_scalar_max(
                                        h1v[:], pre[:], 0.0)
                                else:
                                    nc.scalar.activation(
                                        h1v[:], pre[:],
                                        mybir.ActivationFunctionType.Relu)
                                ndve += 1
                            h1_views[v] = h1v[:]

                # --- W2 + relu(+b2): 12 variants in 6 psum pairs ---
                w2_order = [NAV, 0] + list(range(1, NAV))
                h2_pairs = []
                for p in range(6):
                    va, vb = w2_order[2 * p], w2_order[2 * p + 1]
                    h2ps = ph2.tile([64, 2, F], f32, tag="h2ps")
                    for j, v in enumerate((va, vb)):
                        nc.tensor.matmul(h2ps[:, j, :], w2_s[:], h1_views[v],
                                         start=True, stop=True)
                    h2sb = h2_pool.tile([64, 2, F], f32r)
                    if p % 3 != 2:
                        nc.scalar.activation(h2sb[:], h2ps[:],
                                             mybir.ActivationFunctionType.Relu,
                                             bias=b2_bias)
                    else:
                        nc.vector.tensor_scalar(
                            out=h2sb[:], in0=h2ps[:],
                            scalar1=b2_bias, scalar2=0.0,
                            op0=mybir.AluOpType.add,
                            op1=mybir.AluOpType.max,
                        )
                    h2_pairs.append(h2sb)

                # --- W3 ---
                pwo = pmisc.tile([64, F], f32, tag="misc")
                pwith = pmisc.tile([64, F], f32, tag="misc")
                nc.tensor.matmul(pwo[:], w3d_s[0:64, :], h2_pairs[0][:, 0, :],
                                 start=True, stop=False)
                nc.tensor.matmul(pwith[:], w3d_s[0:64, :], h2_pairs[0][:, 1, :],
                                 start=True, stop=True)
                for p in range(1, 6):
                    h2st = h2st_pool.tile([128, F], f32r)
                    nc.gpsimd.dma_start(out=h2st[0:64, :], in_=h2_pairs[p][:, 0, :])
                    nc.gpsimd.dma_start(out=h2st[64:128, :], in_=h2_pairs[p][:, 1, :])
                    nc.tensor.matmul(pwo[:], w3d_s[:], h2st[:],
                                     start=False, stop=(p == 5))

                # --- exp / d / u ---
                e_sb = eu_pool.tile([64, 2, F], f32r, tag="e")
                nc.scalar.activation(e_sb[:, 0, :], pwo[:],
                                     mybir.ActivationFunctionType.Exp,
                                     bias=b3_bias, scale=1.0 / NWO)
                nc.scalar.activation(e_sb[:, 1, :], pwith[:],
                                     mybir.ActivationFunctionType.Exp,
                                     bias=b3_bias, scale=1.0)
                pwith_sb = eu_pool.tile([64, F], f32r, tag="pw")
                nc.vector.tensor_scalar(
                    out=pwith_sb[:], in0=pwith[:],
                    scalar1=1.0, scalar2=None,
                    op0=mybir.AluOpType.mult,
                )
                d_sb = eu_pool.tile([64, F], f32r, tag="d")
                nc.vector.scalar_tensor_tensor(
                    out=d_sb[:], in0=pwo[:],
                    scalar=1.0 / NWO, in1=pwith_sb[:],
                    op0=mybir.AluOpType.mult,
                    op1=mybir.AluOpType.subtract,
                )
                u_sb = eu_pool.tile([64, F], f32r, tag="u")
                nc.vector.tensor_tensor(
                    out=u_sb[:], in0=d_sb[:], in1=e_sb[:, 0, :],
                    op=mybir.AluOpType.mult,
                )

                # --- per-sample stats (M=1 ones-matmuls) ---
                zq_ps = pmisc.tile([64, F], f32, tag="misc")
                nc.tensor.matmul(zq_ps[0:1, :], redv_s[:, 0:1], e_sb[:, 0, :],
                                 start=True, stop=True)
                zw_ps = pmisc.tile([64, F], f32, tag="misc")
                nc.tensor.matmul(zw_ps[0:1, :], redv_s[:, 0:1], e_sb[:, 1, :],
                                 start=True, stop=True)
                us_ps = pmisc.tile([64, F], f32, tag="misc")
                nc.tensor.matmul(us_ps[0:1, :], redv_s[:, 1:2], u_sb[:],
                                 start=True, stop=True)

                rows = slice(4 * t, 4 * t + 4)
                for k, (ps_t, stage) in enumerate(
                    ((zq_ps, zq_st), (zw_ps, zw_st), (us_ps, u_st))
                ):
                    s_sb = ssb_pool.tile([1, F], f32)
                    if k == 0:
                        nc.vector.tensor_scalar(
                            out=s_sb[:], in0=ps_t[0:1, :],
                            scalar1=1.0, scalar2=None,
                            op0=mybir.AluOpType.mult,
                        )
                    else:
                        nc.scalar.activation(s_sb[:], ps_t[0:1, :],
                                             mybir.ActivationFunctionType.Copy)
                    nc.gpsimd.dma_start(out=stage[rows, :], in_=s_sb[:])

            # --- batched tail: out = (U/64)/Zq + (ln Zw - ln Zq)/64 ---
            rq = tail_pool.tile([128, 128], f32)
            nc.vector.reciprocal(rq[:], zq_st[:])
            lnq = tail_pool.tile([128, 128], f32)
            nc.scalar.activation(lnq[:], zq_st[:], mybir.ActivationFunctionType.Ln)
            lnw = tail_pool.tile([128, 128], f32)
            nc.scalar.activation(lnw[:], zw_st[:], mybir.ActivationFunctionType.Ln)
            d1 = tail_pool.tile([128, 128], f32)
            nc.vector.tensor_tensor(out=d1[:], in0=lnw[:], in1=lnq[:],
                                    op=mybir.AluOpType.subtract)
            t1 = tail_pool.tile([128, 128], f32)
            nc.vector.tensor_tensor(out=t1[:], in0=u_st[:], in1=rq[:],
                                    op=mybir.AluOpType.mult)
            out_sb = tail_pool.tile([128, 128], f32)
            nc.vector.scalar_tensor_tensor(
                out=out_sb[:], in0=d1[:],
                scalar=1.0 / A, in1=t1[:],
                op0=mybir.AluOpType.mult,
                op1=mybir.AluOpType.add,
            )
            nc.sync.dma_start(
                out=out_ap.rearrange("(a b) -> a b", b=128), in_=out_sb[:]
            )

    nc.compile()
    _CACHE["nc"] = nc
    return nc


def _get_cf():
    if "cf" in _CACHE:
        return _CACHE["cf"]
    import jax

    with jax.default_device(jax.devices("cpu")[0]):
        keys = jax.random.split(jax.random.key(42), NCF)
        cf = np.stack(
            [
                np.asarray(jax.random.uniform(k, (B, T, N, A), dtype=np.float32))
                for k in keys
            ]
        )
    _CACHE["cf"] = cf
    return cf


def kernel(obs, actions, W1, b1, W2, b2, W3, b3):
    obs = np.asarray(obs, dtype=np.float32)
    actions = np.asarray(actions, dtype=np.float32)
    W1 = np.asarray(W1, dtype=np.float32)
    b1 = np.asarray(b1, dtype=np.float32)
    W2 = np.asarray(W2, dtype=np.float32)
    b2 = np.asarray(b2, dtype=np.float32)
    W3 = np.asarray(W3, dtype=np.float32)
    b3 = np.asarray(b3, dtype=np.float32)

    _shim_axon_hooks()
    nc = _build()
    cf = _get_cf()  # [10, B, T, N, A]

    obs2 = obs.reshape(NCORES, S, D)
    act2 = actions.reshape(NCORES, S, A)
    cf2 = cf.reshape(NCF, NCORES, S, A)

    wobs = np.concatenate([W1[0:128, :], W1[128:256, :]], axis=1)
    wact = np.concatenate([W1[D:, :], W1[D:, :]], axis=0).astype(AV_NP)
    w3d = np.concatenate([W3, W3], axis=0)
    ident = np.eye(128, dtype=np.float32)
    redv = np.stack(
        [np.ones(64, np.float32), np.full(64, 1.0 / A, np.float32)], axis=1
    )
    biasm = np.zeros((128, 3), np.float32)
    biasm[:, 0] = b1
    biasm[0:64, 1] = b2
    biasm[0:64, 2] = b3

    in_maps = []
    for c in range(NCORES):
        avT = np.empty((NAV, A, S), dtype=AV_NP)
        avT[0] = act2[c].T
        for t in range(NCF):
            avT[1 + t] = cf2[t, c].T
        in_maps.append(
            {
                "obsT": np.ascontiguousarray(obs2[c].T),
                "avT": avT,
                "wobs": wobs,
                "wact": wact,
                "w2": W2,
                "w3d": w3d,
                "ident": ident,
                "redv": redv,
                "bias": biasm,
            }
        )

    trace = bool(int(os.environ.get("KERNEL_TRACE", "0")))
    try:
        res = run_bass_kernel_spmd(
            nc, in_maps, core_ids=list(range(NCORES)), trace=trace
        )
    except Exception:
        # transient device errors (e.g. NRT_EXEC_UNIT_UNRECOVERABLE) clear
        # on retry
        res = run_bass_kernel_spmd(
            nc, in_maps, core_ids=list(range(NCORES)), trace=trace
        )
    _CACHE["last_result"] = res
    out = np.concatenate([r["out"] for r in res.results])
    return out.reshape(B, T, N).astype(np.float32)


# revision 17
# speedup vs baseline: 1.0718x; 1.0718x over previous
"""Trainium2 Bass kernel for nn_EnhancedCausalModel.

Computes influence = mean_A[ softmax(p_wo) * (log_softmax(p_wo) - log_softmax(p_with)) ]
where p_with = MLP([obs, actions]) and p_wo is the average of 11 MLP passes
([obs, 0] plus 10 counterfactual uniform action draws, jax threefry key 42).

Strategy (data-parallel over 8 NeuronCores, batch-sharded):
- Activations are feature-major ("transposed"): SBUF tiles are
  [features(partitions), samples(free)]. Host pre-transposes per-core inputs.
- obs @ W1[:256] is computed ONCE per sample tile. Each of the 11 action
  variants adds its small action matmul; the shared (o1+b1) is added either
  by a PE identity-matmul (then a 1-input relu extraction) or by a DVE
  scalar_tensor_tensor (then a cheap SBUF relu) to balance engine load.
- Layer 3 is linear: the 11 "without" variants accumulate in PSUM. Pairs of
  h2 tensors are stacked into [128, F] via SBUF->SBUF DMA so one K=128
  matmul with [W3; W3] handles two variants.
- Softmax/KL stats (Zq, Zw, U) are per-sample partition sums via M=1
  ones-matmuls; U uses the difference-first formulation
  u = e_wo * (p_wo/11 - p_with) to preserve the tiny output's precision.
- All fp32 matmuls use float32r (full rate on TRN2, ~1.5e-4 per-matmul).
"""

import os
import sys
import types
import numpy as np
import ml_dtypes
from contextlib import ExitStack

import concourse.bass as bass
import concourse.tile as tile
from concourse import bacc, mybir
from concourse.bass_utils import run_bass_kernel_spmd

f32 = mybir.dt.float32
f32r = mybir.dt.float32r
bf16 = mybir.dt.bfloat16

# Problem shape (hardcoded per spec)
B, T, N, D, A = 32, 512, 8, 256, 64
NCORES = 8
S = B * T * N // NCORES  # samples per core = 16384
F = 512                  # samples per tile
NT = S // F              # 32 tiles
H1, H2 = 128, 64
NCF = 10                 # counterfactual draws
NAV = NCF + 1            # action variants (real + cf)
NWO = NCF + 1            # "without" variants averaged (zeros + cf)

AV_BF16 = bool(int(os.environ.get("AV_BF16", "0")))
AV_DT = bf16 if AV_BF16 else f32r
AV_NP = ml_dtypes.bfloat16 if AV_BF16 else np.float32

# Variants 0..10 (0 = real actions). First NIADD go through the PE
# identity-add path; the rest through DVE scalar_tensor_tensor.
NIADD = int(os.environ.get("NIADD", "9"))

_CACHE = {}


def _shim_axon_hooks():
    """The image's antenv lacks axon_hooks; provide it so trace=True works."""
    if "antenv.axon_hooks" in sys.modules:
        return
    try:
        import antenv
        import trn_agent_boot.trn_boot as tb
        import concourse.bass_utils as bu

        mod = types.ModuleType("antenv.axon_hooks")
        _hook = [None]
        mod.set_axon_ntff_profile_hook = lambda h: _hook.__setitem__(0, h)
        mod.get_axon_ntff_profile_hook = lambda: _hook[0]
        sys.modules["antenv.axon_hooks"] = mod
        antenv.axon_hooks = mod
        mod.set_axon_ntff_profile_hook(
            tb._ntff_profile_via_ctypes("/opt/axon/libaxon_pjrt.so")
        )
        bu.upload_artifacts = lambda d: d
    except Exception:
        pass


def _build():
    if "nc" in _CACHE:
        return _CACHE["nc"]

    nc = bacc.Bacc("TRN2", target_bir_lowering=False, debug=False,
                   num_devices=NCORES)

    obsT_ap = nc.dram_tensor("obsT", [D, S], f32r, kind="ExternalInput").ap()
    avT_ap = nc.dram_tensor("avT", [NAV, A, S], AV_DT, kind="ExternalInput").ap()
    wobs_ap = nc.dram_tensor("wobs", [128, D], f32r, kind="ExternalInput").ap()
    wact_ap = nc.dram_tensor("wact", [128, H1], AV_DT, kind="ExternalInput").ap()
    w2_ap = nc.dram_tensor("w2", [H1, H2], f32r, kind="ExternalInput").ap()
    w3d_ap = nc.dram_tensor("w3d", [128, A], f32r, kind="ExternalInput").ap()
    ident_ap = nc.dram_tensor("ident", [128, 128], f32r, kind="ExternalInput").ap()
    redv_ap = nc.dram_tensor("redv", [64, 2], f32r, kind="ExternalInput").ap()
    bias_ap = nc.dram_tensor("bias", [128, 3], f32, kind="ExternalInput").ap()
    out_ap = nc.dram_tensor("out", [S], f32, kind="ExternalOutput").ap()

    with tile.TileContext(nc) as tc:
        with ExitStack() as ctx:
            singles = ctx.enter_context(tc.tile_pool(name="singles", bufs=1))
            obs_pool = ctx.enter_context(tc.tile_pool(name="obs", bufs=4))
            av_pool = ctx.enter_context(tc.tile_pool(name="av", bufs=12))
            sb1 = ctx.enter_context(tc.tile_pool(name="sb1", bufs=3))
            h1_pool = ctx.enter_context(tc.tile_pool(name="h1", bufs=6))
            h2_pool = ctx.enter_context(tc.tile_pool(name="h2", bufs=4))
            h2st_pool = ctx.enter_context(tc.tile_pool(name="h2st", bufs=4))
            eu_pool = ctx.enter_context(tc.tile_pool(name="eu", bufs=2))
            ssb_pool = ctx.enter_context(tc.tile_pool(name="ssb", bufs=4))
            tail_pool = ctx.enter_context(tc.tile_pool(name="tail", bufs=1))

            pa1 = ctx.enter_context(tc.tile_pool(name="pa1", bufs=2, space="PSUM"))
            ph2 = ctx.enter_context(tc.tile_pool(name="ph2", bufs=1, space="PSUM"))
            pmisc = ctx.enter_context(tc.tile_pool(name="pmisc", bufs=2, space="PSUM"))

            # --- constants ---
            wobs_s = singles.tile([128, D], f32r)
            nc.sync.dma_start(out=wobs_s[:], in_=wobs_ap[:])
            wact_s = singles.tile([128, H1], AV_DT)
            nc.sync.dma_start(out=wact_s[:], in_=wact_ap[:])
            w2_s = singles.tile([H1, H2], f32r)
            nc.sync.dma_start(out=w2_s[:], in_=w2_ap[:])
            w3d_s = singles.tile([128, A], f32r)  # [W3; W3]
            nc.sync.dma_start(out=w3d_s[:], in_=w3d_ap[:])
            ident_s = singles.tile([128, 128], f32r)
            nc.sync.dma_start(out=ident_s[:], in_=ident_ap[:])
            redv_s = singles.tile([64, 2], f32r)  # c0=+1, c1=+1/64
            nc.sync.dma_start(out=redv_s[:], in_=redv_ap[:])
            bias_s = singles.tile([128, 3], f32)
            nc.sync.dma_start(out=bias_s[:], in_=bias_ap[:])

            # staging for the batched tail
            zq_st = singles.tile([128, 128], f32)
            zw_st = singles.tile([128, 128], f32)
            u_st = singles.tile([128, 128], f32)

            b1_bias = bias_s[:, 0:1]
            b2_bias = bias_s[0:64, 1:2]
            b3_bias = bias_s[0:64, 2:3]

            obsT_r = obsT_ap.rearrange("(c p) s -> p c s", p=128)

            for t in range(NT):
                ts = bass.ts(t, F)

                # --- loads ---
                obs_t = obs_pool.tile([128, 2, F], f32r)
                nc.sync.dma_start(out=obs_t[:], in_=obsT_r[:, :, ts])
                av_tiles = []
                for p in range((NAV + 1) // 2):
                    avp = av_pool.tile([128, F], AV_DT)
                    nc.sync.dma_start(out=avp[0:64, :], in_=avT_ap[2 * p, :, ts])
                    if 2 * p + 1 < NAV:
                        nc.sync.dma_start(
                            out=avp[64:128, :], in_=avT_ap[2 * p + 1, :, ts]
                        )
                    av_tiles.append(avp)

                # --- o1 = W1o.T @ obsT  (shared) ---
                o1t = pa1.tile([128, 2, F], f32, tag="a1")
                o1ps = o1t[:, 0, :]
                nc.tensor.matmul(o1ps, wobs_s[:, 0:128], obs_t[:, 0, :],
                                 start=True, stop=False)
                nc.tensor.matmul(o1ps, wobs_s[:, 128:256], obs_t[:, 1, :],
                                 start=False, stop=True)

                # o1b = o1 + b1 -> SBUF (f32r)
                o1b = sb1.tile([128, F], f32r)
                nc.scalar.activation(o1b[:], o1ps,
                                     mybir.ActivationFunctionType.Identity,
                                     bias=b1_bias)

                # --- h1 for 12 variants ---
                h1_views = [None] * (NAV + 1)  # 0..10 variants, 11 = zeros

                h1z = h1_pool.tile([128, F], f32r, tag="h1s")
                nc.vector.tensor_scalar_max(h1z[:], o1b[:], 0.0)
                h1_views[NAV] = h1z[:]

                ndve = 0
                for p in range((NAV + 1) // 2):
                    vs = [v for v in (2 * p, 2 * p + 1) if v < NAV]
                    paps = pa1.tile([128, 2, F], f32, tag="a1")
                    # a1 matmuls first (row-strip pair overlaps), then the
                    # identity-adds back to back (same weights adjacent).
                    for j, v in enumerate(vs):
                        nc.tensor.matmul(
                            paps[:, j, :],
                            wact_s[64 * j: 64 * (j + 1), :],
                            av_tiles[p][64 * j: 64 * (j + 1), :],
                            start=True,
                            stop=not (v < NIADD),
                        )
                    for j, v in enumerate(vs):
                        if v < NIADD:
                            nc.tensor.matmul(paps[:, j, :], ident_s[:], o1b[:],
                                             start=False, stop=True)
                    if all(v < NIADD for v in vs) and len(vs) == 2:
                        # paired 1-input relu extraction
                        h1p = h1_pool.tile([128, 2, F], f32r, tag="h1p")
                        if p % 2 == 0:
                            nc.scalar.activation(
                                h1p[:], paps[:],
                                mybir.ActivationFunctionType.Relu)
                        else:
                            nc.vector.tensor_scalar(
                                out=h1p[:], in0=paps[:],
                                scalar1=0.0, scalar2=None,
                                op0=mybir.AluOpType.max,
                            )
                        for j, v in enumerate(vs):
                            h1_views[v] = h1p[:, j, :]
                    else:
                        for j, v in enumerate(vs):
                            h1v = h1_pool.tile([128, F], f32r, tag="h1s")
                            if v < NIADD:
                                nc.scalar.activation(
                                    h1v[:], paps[:, j, :],
                                    mybir.ActivationFunctionType.Relu)
                            else:
                                # DVE add of o1b, then cheap SBUF relu
                                pre = h1_pool.tile([128, F], f32r, tag="h1pre")
                                nc.vector.scalar_tensor_tensor(
                                    out=pre[:], in0=paps[:, j, :],
                                    scalar=1.0, in1=o1b[:],
                                    op0=mybir.AluOpType.mult,
                                    op1=mybir.AluOpType.add,
                                )
                                if ndve % 2 == 0:
                                    nc.vector.tensor_scalar_max(
                                        h1v[:], pre[:], 0.0)
                                else:
                                    nc.scalar.activation(
                                        h1v[:], pre[:],
                                        mybir.ActivationFunctionType.Relu)
                                ndve += 1
                            h1_views[v] = h1v[:]

                # --- W2 + relu(+b2): 12 variants in 6 psum pairs ---
                w2_order = [NAV, 0] + list(range(1, NAV))
                h2_pairs = []
                for p in range(6):
                    va, vb = w2_order[2 * p], w2_order[2 * p + 1]
                    h2ps = ph2.tile([64, 2, F], f32, tag="h2ps")
                    for j, v in enumerate((va, vb)):
                        nc.tensor.matmul(h2ps[:, j, :], w2_s[:], h1_views[v],
                                         start=True, stop=True)
                    h2sb = h2_pool.tile([64, 2, F], f32r)
                    if p % 3 != 2:
                        nc.scalar.activation(h2sb[:], h2ps[:],
                                             mybir.ActivationFunctionType.Relu,
                                             bias=b2_bias)
                    else:
                        nc.vector.tensor_scalar(
                            out=h2sb[:], in0=h2ps[:],
                            scalar1=b2_bias, scalar2=0.0,
                            op0=mybir.AluOpType.add,
                            op1=mybir.AluOpType.max,
                        )
                    h2_pairs.append(h2sb)

                # --- W3 ---
                pwo = pmisc.tile([64, F], f32, tag="misc")
                pwith = pmisc.tile([64, F], f32, tag="misc")
                nc.tensor.matmul(pwo[:], w3d_s[0:64, :], h2_pairs[0][:, 0, :],
                                 start=True, stop=False)
                nc.tensor.matmul(pwith[:], w3d_s[0:64, :], h2_pairs[0][:, 1, :],
                                 start=True, stop=True)
                for p in range(1, 6):
                    h2st = h2st_pool.tile([128, F], f32r)
                    nc.gpsimd.dma_start(out=h2st[0:64, :], in_=h2_pairs[p][:, 0, :])
                    nc.gpsimd.dma_start(out=h2st[64:128, :], in_=h2_pairs[p][:, 1, :])
                    nc.tensor.matmul(pwo[:], w3d_s[:], h2st[:],
                                     start=False, stop=(p == 5))

                # --- exp / d / u ---
                e_sb = eu_pool.tile([64, 2, F], f32r, tag="e")
                nc.scalar.activation(e_sb[:, 0, :], pwo[:],
                                     mybir.ActivationFunctionType.Exp,
                                     bias=b3_bias, scale=1.0 / NWO)
                nc.scalar.activation(e_sb[:, 1, :], pwith[:],
                                     mybir.ActivationFunctionType.Exp,
                                     bias=b3_bias, scale=1.0)
                pwith_sb = eu_pool.tile([64, F], f32r, tag="pw")
                nc.vector.tensor_scalar(
                    out=pwith_sb[:], in0=pwith[:],
                    scalar1=1.0, scalar2=None,
                    op0=mybir.AluOpType.mult,
                )
                d_sb = eu_pool.tile([64, F], f32r, tag="d")
                nc.vector.scalar_tensor_tensor(
                    out=d_sb[:], in0=pwo[:],
                    scalar=1.0 / NWO, in1=pwith_sb[:],
                    op0=mybir.AluOpType.mult,
                    op1=mybir.AluOpType.subtract,
                )
                u_sb = eu_pool.tile([64, F], f32r, tag="u")
                nc.vector.tensor_tensor(
                    out=u_sb[:], in0=d_sb[:], in1=e_sb[:, 0, :],
                    op=mybir.AluOpType.mult,
                )

                # --- per-sample stats (M=1 ones-matmuls) ---
                zq_ps = pmisc.tile([64, F], f32, tag="misc")
                nc.tensor.matmul(zq_ps[0:1, :], redv_s[:, 0:1], e_sb[:, 0, :],
                                 start=True, stop=True)
                zw_ps = pmisc.tile([64, F], f32, tag="misc")
                nc.tensor.matmul(zw_ps[0:1, :], redv_s[:, 0:1], e_sb[:, 1, :],
                                 start=True, stop=True)
                us_ps = pmisc.tile([64, F], f32, tag="misc")
                nc.tensor.matmul(us_ps[0:1, :], redv_s[:, 1:2], u_sb[:],
                                 start=True, stop=True)

                rows = slice(4 * t, 4 * t + 4)
                for k, (ps_t, stage) in enumerate(
                    ((zq_ps, zq_st), (zw_ps, zw_st), (us_ps, u_st))
                ):
                    s_sb = ssb_pool.tile([1, F], f32)
                    if k == 0:
                        nc.vector.tensor_scalar(
                            out=s_sb[:], in0=ps_t[0:1, :],
                            scalar1=1.0, scalar2=None,
                            op0=mybir.AluOpType.mult,
                        )
                    else:
                        nc.scalar.activation(s_sb[:], ps_t[0:1, :],
                                             mybir.ActivationFunctionType.Copy)
                    nc.gpsimd.dma_start(out=stage[rows, :], in_=s_sb[:])

            # --- batched tail: out = (U/64)/Zq + (ln Zw - ln Zq)/64 ---
            rq = tail_pool.tile([128, 128], f32)
            nc.vector.reciprocal(rq[:], zq_st[:])
            lnq = tail_pool.tile([128, 128], f32)
            nc.scalar.activation(lnq[:], zq_st[:], mybir.ActivationFunctionType.Ln)
            lnw = tail_pool.tile([128, 128], f32)
            nc.scalar.activation(lnw[:], zw_st[:], mybir.ActivationFunctionType.Ln)
            d1 = tail_pool.tile([128, 128], f32)
            nc.vector.tensor_tensor(out=d1[:], in0=lnw[:], in1=lnq[:],
                                    op=mybir.AluOpType.subtract)
            t1 = tail_pool.tile([128, 128], f32)
            nc.vector.tensor_tensor(out=t1[:], in0=u_st[:], in1=rq[:],
                                    op=mybir.AluOpType.mult)
            out_sb = tail_pool.tile([128, 128], f32)
            nc.vector.scalar_tensor_tensor(
                out=out_sb[:], in0=d1[:],
                scalar=1.0 / A, in1=t1[:],
                op0=mybir.AluOpType.mult,
                op1=mybir.AluOpType.add,
            )
            nc.sync.dma_start(
                out=out_ap.rearrange("(a b) -> a b", b=128), in_=out_sb[:]
            )

    nc.compile()
    _CACHE["nc"] = nc
    return nc


def _get_cf():
    if "cf" in _CACHE:
        return _CACHE["cf"]
    import jax

    with jax.default_device(jax.devices("cpu")[0]):
        keys = jax.random.split(jax.random.key(42), NCF)
        cf = np.stack(
            [
                np.asarray(jax.random.uniform(k, (B, T, N, A), dtype=np.float32))
                for k in keys
            ]
        )
    _CACHE["cf"] = cf
    return cf


def kernel(obs, actions, W1, b1, W2, b2, W3, b3):
    obs = np.asarray(obs, dtype=np.float32)
    actions = np.asarray(actions, dtype=np.float32)
    W1 = np.asarray(W1, dtype=np.float32)
    b1 = np.asarray(b1, dtype=np.float32)
    W2 = np.asarray(W2, dtype=np.float32)
    b2 = np.asarray(b2, dtype=np.float32)
    W3 = np.asarray(W3, dtype=np.float32)
    b3 = np.asarray(b3, dtype=np.float32)

    _shim_axon_hooks()
    nc = _build()
    cf = _get_cf()  # [10, B, T, N, A]

    obs2 = obs.reshape(NCORES, S, D)
    act2 = actions.reshape(NCORES, S, A)
    cf2 = cf.reshape(NCF, NCORES, S, A)

    wobs = np.concatenate([W1[0:128, :], W1[128:256, :]], axis=1)
    wact = np.concatenate([W1[D:, :], W1[D:, :]], axis=0).astype(AV_NP)
    w3d = np.concatenate([W3, W3], axis=0)
    ident = np.eye(128, dtype=np.float32)
    redv = np.stack(
        [np.ones(64, np.float32), np.full(64, 1.0 / A, np.float32)], axis=1
    )
    biasm = np.zeros((128, 3), np.float32)
    biasm[:, 0] = b1
    biasm[0:64, 1] = b2
    biasm[0:64, 2] = b3

    in_maps = []
    for c in range(NCORES):
        avT = np.empty((NAV, A, S), dtype=AV_NP)
        avT[0] = act2[c].T
        for t in range(NCF):
            avT[1 + t] = cf2[t, c].T
        in_maps.append(
            {
                "obsT": np.ascontiguousarray(obs2[c].T),
                "avT": avT,
                "wobs": wobs,
                "wact": wact,
                "w2": W2,
                "w3d": w3d,
                "ident": ident,
                "redv": redv,
                "bias": biasm,
            }
        )

    trace = bool(int(os.environ.get("KERNEL_TRACE", "0")))
    try:
        res = run_bass_kernel_spmd(
            nc, in_maps, core_ids=list(range(NCORES)), trace=trace
        )
    except Exception:
        # transient device errors (e.g. NRT_EXEC_UNIT_UNRECOVERABLE) clear
        # on retry
        res = run_bass_kernel_spmd(
            nc, in_maps, core_ids=list(range(NCORES)), trace=trace
        )
    _CACHE["last_result"] = res
    out = np.concatenate([r["out"] for r in res.results])
    return out.reshape(B, T, N).astype(np.float32)
